# revision 17
# baseline (speedup 1.0000x reference)
"""Multi-head self-attention Trainium2 kernel (8 NeuronCores, batch-parallel).

Reference: qkv = x @ W_qkv + b; 12-head scaled-dot-product attention; concat.
Shapes: x[8,1024,768], W_qkv[768,2304], b_qkv[2304] -> out[8,1024,768].
Sharding: one batch element per core; W/b replicated to all cores.

Per-core dataflow:
  x --PE transpose--> xT[768,1024] (f32r), copies batched 4 chunks at a time
  qk tiles (bf16): per (f-block, token-half) [128,512] = W-block(lhsT) @ xT
    produced in N>=256 slices; Q/K biases added on the PSUM->SBUF copy
  V[128,12,66] bf16 per token chunk (strips of 4 heads; col 64 = ones)
  per (pair p, q-half qh), per key-chunk kc:
    scT[128,2,512] = K-slice(lhsT) @ Q-half  (2 row-tiled MMs, one per head)
    ex[128,2,512] bf16 = ACT Exp(0.125 * scT)   (scale folded into ACT)
    av[q=128,65] += ex-chunk(lhsT) @ [V_h|1]  bf16 N=65 MMs, accumulated
      over kc; av already in [q, feature] orientation, col 64 = denominator
  normalize: rc = 1/av[:,:,64] (DVE), onat[:, c, h*64:...] = av * rc
  out DMA per chunk once the last pair finishes its q-half.

Scheduling: W is DMA'd in priority order (pair-0 Q/K columns, V strip 0,
then later pairs); QK-tile and V-strip production is spread across the
pair loop just-in-time so PE work per key-chunk stays balanced against
the ACT exp stream (ACT is the co-bottleneck at ~1038ns per key-chunk).
"""

import contextlib
import json as _json

import numpy as np

import concourse.bass as bass
import concourse.mybir as mybir
import concourse.tile as tile
from concourse.bass_utils import run_bass_kernel_spmd
from concourse.masks import make_identity

# --- BIR sync-wait legalization ------------------------------------------
# walrus's codegen in this toolchain accepts only one sync-wait command per
# instruction. Split every multi-wait instruction into N-1 preceding
# single-wait EventSemaphore instructions on the same engine.


def _legalize_sync_waits(bir_json: bytes) -> bytes:
    m = _json.loads(bir_json)
    ctr = 0
    for fn in m["functions"]:
        for bb in fn["blocks"]:
            out = []
            for ins in bb["instructions"]:
                si = ins.get("sync_info")
                waits = si.get("on_wait", []) if si else []
                if len(waits) > 1:
                    for w in waits[:-1]:
                        ctr += 1
                        out.append(
                            {
                                "debug": ins.get("debug", 0),
                                "engine": ins["engine"],
                                "ins": [],
                                "outs": [],
                                "name": f"evw-split-{ctr}",
                                "opcode": "EventSemaphore",
                                "sync_info": {"on_update": [], "on_wait": [w]},
                            }
                        )
                    si["on_wait"] = [waits[-1]]
                out.append(ins)
            bb["instructions"] = out
    return _json.dumps(m).encode()


_fixup_installed = False


def _install_bir_fixup():
    global _fixup_installed
    if _fixup_installed:
        return
    _fixup_installed = True
    import concourse.bass_utils as _bu

    _orig = _bu.compile_bir_kernel

    def _patched(bir_json, tmpdir, neff_name="file.neff"):
        if isinstance(bir_json, str):
            bir_json = bir_json.encode()
        return _orig(_legalize_sync_waits(bir_json), tmpdir, neff_name)

    _bu.compile_bir_kernel = _patched
    try:
        import concourse.bass2jax as _b2j

        _b2j.compile_bir_kernel = _patched
    except ImportError:
        pass


_install_bir_fixup()

B, N, D, H = 8, 1024, 768, 12
HD = D // H            # 64
F3 = 3 * D             # 2304
NCORE = 8
P = 128
NCHUNK = N // P        # 8 token chunks
KD = D // P            # 6 d_in chunks
QH = 512               # q-half size
NPAIR = H // 2         # 6
VW = HD + 1            # 65 (V cols + denominator ones col)
VPAD = 66              # padded per-head V width (4-byte aligned bf16)

f32 = mybir.dt.float32
f32r = mybir.dt.float32r
bf16 = mybir.dt.bfloat16
fp8 = mybir.dt.float8e4
FT = mybir.ActivationFunctionType
ALU = mybir.AluOpType
DR = mybir.MatmulPerfMode.DoubleRow

# fp8 scores were tried and rejected: DoubleRow's AP layout crashed the
# device for Ki=32, and fp8e4m3's ~3.6%/operand quantization error puts the
# output at ~1-2e-2 — too close to the 2e-2 gate. Keep bf16.
SCORES_FP8 = False


def build_attention_nc():
    nc = bass.Bass()
    x_d = nc.declare_dram_parameter("x", [N, D], f32, isOutput=False)
    w_d = nc.declare_dram_parameter("W_qkv", [D, F3], f32, isOutput=False)
    b_d = nc.declare_dram_parameter("b_qkv", [F3], f32, isOutput=False)
    o_d = nc.declare_dram_parameter("out", [N, D], f32, isOutput=True)

    with tile.TileContext(nc) as tc, contextlib.ExitStack() as ctx:
        singles = ctx.enter_context(tc.tile_pool(name="singles", bufs=1))
        qkpool = ctx.enter_context(tc.tile_pool(name="qkpool", bufs=10))
        vpool = ctx.enter_context(tc.tile_pool(name="vpool", bufs=NCHUNK))
        exppool = ctx.enter_context(tc.tile_pool(name="exppool", bufs=5))
        recpool = ctx.enter_context(tc.tile_pool(name="recpool", bufs=4))

        # PSUM budget (8 banks): wk [128,512] x2 = 2; sc [128,2,512] x2 = 4;
        # av [128,4,66] x2 = 2.
        wkps = ctx.enter_context(tc.tile_pool(name="wkps", bufs=2, space="PSUM"))
        scps = ctx.enter_context(tc.tile_pool(name="scps", bufs=2, space="PSUM"))
        avps = ctx.enter_context(tc.tile_pool(name="avps", bufs=2, space="PSUM"))

        def wk_psum():
            return wkps.tile([P, QH], f32, tag="wk", name="wktile")

        # ------------- constants -------------------------------------------
        ident = singles.tile([P, P], f32)
        make_identity(nc, ident)  # gpsimd

        ident_r = singles.tile([P, P], f32r)
        nc.vector.tensor_copy(out=ident_r, in_=ident)

        ones_f32 = singles.tile([P, 1], f32)
        nc.vector.memset(ones_f32, 1.0)
        ones_row_st = singles.tile([1, P], f32)
        nc.vector.memset(ones_row_st, 1.0)
        ones_row = singles.tile([1, P], f32r)
        nc.vector.tensor_copy(out=ones_row, in_=ones_row_st)

        # dummy exp to trigger the ACT table load early
        actwarm = singles.tile([1, 2], f32)
        nc.vector.memset(actwarm, 0.0)
        nc.scalar.activation(actwarm, actwarm, FT.Exp)

        bv_st = singles.tile([1, D], f32)
        nc.sync.dma_start(out=bv_st, in_=b_d[2 * D : 3 * D][None, :])
        bv_sb = singles.tile([1, D], f32r)
        nc.vector.tensor_copy(out=bv_sb, in_=bv_st)

        # ------------- input DMAs (batched, priority order) ----------------
        # HWDGE charges a flat ~625ns per DMA instruction, serialized — so
        # batch: one DMA per W column block covering all 6 k-chunks, and
        # 2-chunk x DMAs.
        x_big = singles.tile([P, NCHUNK, D], f32r)
        x_sb = [x_big[:, c, :] for c in range(NCHUNK)]

        def dma_x(c0, nc_=2):
            nc.sync.dma_start(
                out=x_big[:, c0 : c0 + nc_, :],
                in_=x_d[c0 * P : (c0 + nc_) * P, :]
                .bitcast(f32r)
                .rearrange("(c p) d -> p c d", p=P),
            )

        w_big = singles.tile([P, KD, F3], f32r)
        w_sb = [w_big[:, k, :] for k in range(KD)]

        def dma_w_cols(f0, fw):
            nc.sync.dma_start(
                out=w_big[:, :, f0 : f0 + fw],
                in_=w_d[:, f0 : f0 + fw]
                .bitcast(f32r)
                .rearrange("(k p) f -> p k f", p=P),
            )

        # single-chunk x DMAs at the start so transposes begin ASAP and the
        # PE stays continuously busy through its ramp-up; K cols + bias
        # before Q cols so kt production (needing only x0,x1) starts first
        dma_x(0, 1)
        dma_x(1, 1)
        dma_w_cols(6 * P, P)          # pair-0 K cols
        b_sb = singles.tile([P, 2 * KD], f32)  # Q/K biases only; V uses bv
        nc.sync.dma_start(
            out=b_sb, in_=b_d[0 : 2 * D].rearrange("(t p) -> p t", p=P)
        )
        dma_w_cols(0 * P, P)          # pair-0 Q cols
        dma_x(2, 1)
        dma_x(3, 1)
        dma_x(4, 1)
        dma_x(5, 1)
        dma_w_cols(2 * D, 2 * P)      # V strip 0 (heads 0-3)
        dma_x(6)
        dma_w_cols(1 * P, P)          # pair-1 Q
        dma_w_cols(7 * P, P)          # pair-1 K
        dma_w_cols(2 * D + 2 * P, 2 * P)   # V strip 1 (heads 4-7)
        dma_w_cols(2 * P, P)
        dma_w_cols(8 * P, P)
        dma_w_cols(2 * D + 4 * P, 2 * P)   # V strip 2 (heads 8-11)
        for p in range(3, NPAIR):
            dma_w_cols(p * P, P)
            dma_w_cols((6 + p) * P, P)

        # ------------- x^T (PE transposes, batched copies) ------------------
        # xt is one [P, KD, N] tile so a chunk's transposes for several
        # k-slices drain through a single strided DVE copy
        xt_big = singles.tile([P, KD, N], f32r)
        xt = [xt_big[:, k, :] for k in range(KD)]

        def transpose_chunk(c):
            # transpose x chunk c into xt[k][:, c*P:(c+1)*P] for all k
            for k0, kn in ((0, 4), (4, 2)):
                ps = wk_psum()[:, 0 : kn * P]
                for j in range(kn):
                    nc.tensor.transpose(
                        ps[:, j * P : (j + 1) * P].bitcast(f32r),
                        x_sb[c][:, (k0 + j) * P : (k0 + j + 1) * P],
                        ident_r,
                    )
                nc.vector.tensor_copy(
                    out=xt_big[:, k0 : k0 + kn, c * P : (c + 1) * P],
                    in_=ps.rearrange("p (k q) -> p k q", q=P).bitcast(f32r),
                )

        # broadcast b_v across partitions once: bvb[p, f] = b_v[f]
        bvb = singles.tile([P, D], f32)
        for f0 in range(0, D, 256):
            ps = wk_psum()[:, 0:256]
            nc.tensor.matmul(
                ps, ones_row, bv_sb[:, f0 : f0 + 256], start=True, stop=True
            )
            nc.vector.tensor_copy(out=bvb[:, f0 : f0 + 256], in_=ps)

        # ------------- qk tiles ---------------------------------------------
        # qk[(f, half)]: [128, 512] bf16; partitions = features f*128..+128,
        # cols = tokens half*512..+512. f 0..5 = Q blocks, 6..11 = K blocks.
        qk_tiles = {}
        qk8_tiles = {}
        qk_dt = fp8 if SCORES_FP8 else bf16

        def get_qk(f, half):
            key = (f, half)
            if key not in qk_tiles:
                qk_tiles[key] = qkpool.tile(
                    [P, QH], qk_dt, tag="qk", name=f"qk{f}_{half}"
                )
            return qk_tiles[key]

        def get_qk8(f, half):
            # DoubleRow layout: partition 32*hi+p', free (g, tok) holds
            # feature 64*hi + 32*g + p' of block f
            key = (f, half)
            if key not in qk8_tiles:
                qk8_tiles[key] = qkpool.tile(
                    [64, 2, QH], fp8, tag="qk8", name=f"qk8_{f}_{half}"
                )
            return qk8_tiles[key]

        def make_qk(f, half, n0=0, nw=QH, ks=0, ke=KD, _ps=[None]):
            # produce token-cols [n0, n0+nw) of tile (f, half); nw >= 256.
            # ks/ke allow k-chunk-split emission (jit pacing); the PSUM tile
            # is carried across the split via _ps.
            t = get_qk(f, half)
            if ks == 0:
                _ps[0] = wk_psum()[:, 0:nw]
            ps = _ps[0]
            for k in range(ks, ke):
                nc.tensor.matmul(
                    ps,
                    w_sb[k][:, f * P : (f + 1) * P],
                    xt[k][:, half * QH + n0 : half * QH + n0 + nw],
                    start=(k == 0),
                    stop=(k == KD - 1),
                )
            if ke == KD:
                nc.vector.tensor_scalar_add(
                    t[:, n0 : n0 + nw], ps, b_sb[:, f : f + 1]
                )
                if SCORES_FP8:
                    # cross-partition remap into the DoubleRow layout:
                    # out(32*hi+p', g, n) <- t(64*hi + 32*g + p', n)
                    t8 = get_qk8(f, half)
                    for hi in range(2):
                        nc.sync.dma_start(
                            out=t8[32 * hi : 32 * hi + 32, :, n0 : n0 + nw],
                            in_=t[64 * hi : 64 * hi + 64, n0 : n0 + nw]
                            .rearrange("(g q) n -> q g n", g=2),
                        )
            return t

        def qk_halves(f, half):
            # two pacing thunks producing tile (f, half) split by k-chunks
            return [
                lambda: make_qk(f, half, ks=0, ke=3),
                lambda: make_qk(f, half, ks=3, ke=KD),
            ]

        # ------------- V tiles ----------------------------------------------
        # v[c]: [128, 12, 66] bf16; [:, h, 0:64] = V for head h, [:, h, 64] = 1
        v_sb = []
        for c in range(NCHUNK):
            t = vpool.tile([P, H, VPAD], bf16, tag="v", name=f"v{c}")
            v_sb.append(t)

        def make_v(c, s):
            # strip s covers heads 4s..4s+4 (f-cols 2D + s*256 ..+256)
            if s == 0:
                nc.vector.tensor_copy(
                    out=v_sb[c][:, :, HD : HD + 1],
                    in_=ones_f32[:, 0:1, None].to_broadcast([P, H, 1]),
                )
            f0 = s * 256
            ps = wk_psum()[:, 0:256]
            for k in range(KD):
                nc.tensor.matmul(
                    ps,
                    xt[k][:, c * P : (c + 1) * P],
                    w_sb[k][:, 2 * D + f0 : 2 * D + f0 + 256],
                    start=(k == 0),
                    stop=(k == KD - 1),
                )
            nc.vector.tensor_tensor(
                v_sb[c][:, 4 * s : 4 * s + 4, 0:HD],
                ps.rearrange("p (h d) -> p h d", d=HD),
                bvb[:, f0 : f0 + 256].rearrange("p (h d) -> p h d", d=HD),
                ALU.add,
            )

        # ------------- bootstrap: transposes + first tiles ------------------
        # The first scores/exp left half (q tokens 0:256, needing only x0/x1)
        # is emitted mid-bootstrap so the ACT stream starts before x2/x3 land.
        transpose_chunk(0)
        transpose_chunk(1)
        make_qk(6, 0, 0, 256)           # kt(pair0) tokens 0:256
        make_qk(0, 0, 0, 256)           # qt(pair0,qh0) left half
        boot_sc = scps.tile([P, 2, QH], f32, tag="sc", name="sc")
        boot_ex = exppool.tile([P, 2, QH], bf16, tag="exp", name="ex")
        for hi in range(2):
            nc.tensor.matmul(
                boot_sc[:, hi, 0:256],
                get_qk(6, 0)[64 * hi : 64 * hi + 64, 0:P],
                get_qk(0, 0)[64 * hi : 64 * hi + 64, 0:256],
                start=True,
                stop=True,
                tile_position=(64 * hi, 0),
            )
        nc.scalar.activation(
            boot_ex[:, :, 0:256], boot_sc[:, :, 0:256], FT.Exp, scale=0.125
        )
        transpose_chunk(2)
        transpose_chunk(3)
        make_qk(0, 0, 256, 256)         # qt(pair0,qh0) right half

        onat = singles.tile([P, NCHUNK, D], f32)

        # pair-0-qh0 slots are handcrafted against DMA arrival order (jit
        # runs before each iteration's sc MMs, so an item may produce the
        # tile that same iteration reads).
        slot_items = {
            0: [lambda: make_qk(6, 0, 256, 256)],
            1: [lambda: transpose_chunk(4)],
            2: [lambda: transpose_chunk(5)],
            3: [lambda: make_qk(6, 1, 0, 256)],
            4: [lambda: transpose_chunk(6)],
            5: [lambda: transpose_chunk(7)],
            6: [lambda: make_qk(6, 1, 256, 256)],
            7: [qk_halves(0, 1)[0]],
            8: [qk_halves(0, 1)[1]],
        }

        # Remaining production work is placed in the LATEST legal slot
        # (backward greedy by deadline) so the per-iteration PE load is
        # spread toward uniform: the tail pairs have no natural projection
        # work and would otherwise leave the PE idle while ACT catches up.
        items = []  # (deadline_slot, thunk)
        for p in range(1, NPAIR):
            for half in range(2):
                for th in qk_halves(6 + p, half):
                    items.append((16 * p + 4 * half, th))
            for qh in range(2):
                for th in qk_halves(p, qh):
                    items.append((16 * p + 8 * qh, th))
        for s in (1, 2):
            for c in range(NCHUNK):
                items.append((32 * s + c, lambda c=c, s=s: make_v(c, s)))

        free_slots = [s for s in range(9, 16 * NPAIR) if s not in slot_items]
        for s in reversed(free_slots):
            best = None
            for i, (dl, th) in enumerate(items):
                # ties pick the later list index so split-production halves
                # keep their emission order (h1 before h2)
                if dl >= s and (best is None or dl >= items[best][0]):
                    best = i
            if best is not None:
                slot_items[s] = [items.pop(best)[1]]
        for s in reversed(free_slots):  # anything left: earliest slots
            if s not in slot_items and items:
                slot_items[s] = [items.pop()[1]]
        assert not items, f"unscheduled jit items: {len(items)}"

        # ------------- attention pair loop ----------------------------------
        def normalize(p, qh, av):
            # rc = 1/denominator, onat = av * rc
            for hi in range(2):
                h = 2 * p + hi
                rc = recpool.tile([P, 4], f32, tag="rec", name="rc")
                nc.vector.reciprocal(out=rc, in_=av[hi][:, :, HD])
                nc.vector.tensor_tensor(
                    onat[:, qh * 4 : (qh + 1) * 4, h * HD : (h + 1) * HD],
                    av[hi][:, :, 0:HD],
                    rc[:, :, None].to_broadcast([P, 4, HD]),
                    ALU.mult,
                )
                if p == NPAIR - 1 and qh == 1:
                    # final-tail head: DMA its 64 columns immediately
                    nc.sync.dma_start(
                        out=o_d[4 * P : NCHUNK * P, h * HD : (h + 1) * HD]
                        .rearrange("(c p) f -> p c f", p=P),
                        in_=onat[:, 4:NCHUNK, h * HD : (h + 1) * HD],
                    )
            if p == NPAIR - 2 and qh == 1:
                # heads 0-9 of chunks 4-7 are final: DMA them now so only
                # the last pair's 128 columns remain for the tail
                nc.sync.dma_start(
                    out=o_d[4 * P : NCHUNK * P, 0 : 5 * P].rearrange(
                        "(c p) f -> p c f", p=P
                    ),
                    in_=onat[:, 4:NCHUNK, 0 : 5 * P],
                )
            if p == NPAIR - 1 and qh == 0:
                nc.sync.dma_start(
                    out=o_d[0 : 4 * P, :].rearrange("(c p) f -> p c f", p=P),
                    in_=onat[:, 0:4, :],
                )

        # software pipeline: AV for iteration i emitted during iteration i+1,
        # so the next sc MMs (and the exp they feed) aren't serialized behind
        # the AV tail at q-half boundaries.
        stream = [
            (p, qh, kc)
            for p in range(NPAIR)
            for qh in range(2)
            for kc in range(NCHUNK)
        ]
        av_cur = None
        deferred = None  # (p, qh, kc, av, ex) AV work from previous iteration

        def emit_av(p, qh, kc, av, ex):
            for hi in range(2):
                for qc in range(4):
                    # one bank-clearing start per av bank: later first-writes
                    # overwrite per-element (has_written cleared by the
                    # start), later kc's accumulate
                    nc.tensor.matmul(
                        av[hi][:, qc, 0:VW],
                        ex[:, hi, qc * P : (qc + 1) * P],
                        v_sb[kc][:, 2 * p + hi, 0:VW],
                        start=(kc == 0 and qc == 0),
                        stop=(kc == NCHUNK - 1 and qc == 3),
                        skip_group_check=True,
                    )

        for it, (p, qh, kc) in enumerate(stream):
            for th in slot_items.get(it, ()):
                th()
            kcol = (kc % 4) * P
            if it == 0:
                # finish iteration 0: right half of scores + exp
                qt = get_qk(0, 0)
                for hi in range(2):
                    nc.tensor.matmul(
                        boot_sc[:, hi, 256:512],
                        get_qk(6, 0)[64 * hi : 64 * hi + 64, 0:P],
                        qt[64 * hi : 64 * hi + 64, 256:512],
                        start=True,
                        stop=True,
                        tile_position=(64 * hi, 0),
                    )
                make_v(0, 0)
                nc.scalar.activation(
                    boot_ex[:, :, 256:512], boot_sc[:, :, 256:512],
                    FT.Exp, scale=0.125,
                )
                av_cur = [
                    avps.tile([P, 4, VPAD], f32, tag="av", name=f"av{hi}")
                    for hi in range(2)
                ]
                deferred = (p, qh, kc, av_cur, boot_ex)
                continue
            sc = scps.tile([P, 2, QH], f32, tag="sc", name="sc")
            if SCORES_FP8:
                qt8 = get_qk8(p, qh)
                kt8 = get_qk8(6 + p, kc // 4)
                for hi in range(2):
                    nc.tensor.matmul(
                        sc[:, hi, :],
                        kt8[32 * hi : 32 * hi + 32, :, kcol : kcol + P],
                        qt8[32 * hi : 32 * hi + 32, :, :],
                        start=True,
                        stop=True,
                        perf_mode=DR,
                    )
            else:
                qt = get_qk(p, qh)
                kth = get_qk(6 + p, kc // 4)
                for hi in range(2):
                    nc.tensor.matmul(
                        sc[:, hi, :],
                        kth[64 * hi : 64 * hi + 64, kcol : kcol + P],
                        qt[64 * hi : 64 * hi + 64, :],
                        start=True,
                        stop=True,
                        tile_position=(64 * hi, 0),
                    )
            if p == 0 and qh == 0:
                make_v(kc, 0)  # strip 0 JIT (AV needs it next iteration)
            ex = exppool.tile([P, 2, QH], bf16, tag="exp", name="ex")
            nc.scalar.activation(ex, sc, FT.Exp, scale=0.125)
            if deferred is not None:
                emit_av(*deferred)
                dp, dqh, dkc, dav, _ = deferred
                if dkc == NCHUNK - 1:
                    normalize(dp, dqh, dav)
            if kc == 0:
                av_cur = [
                    avps.tile([P, 4, VPAD], f32, tag="av", name=f"av{hi}")
                    for hi in range(2)
                ]
            deferred = (p, qh, kc, av_cur, ex)
        emit_av(*deferred)
        normalize(NPAIR - 1, 1, av_cur)

    return nc


def kernel(x: np.ndarray, W_qkv: np.ndarray, b_qkv: np.ndarray) -> np.ndarray:
    nc = build_attention_nc()
    in_maps = [
        {
            "x": np.ascontiguousarray(x[c], dtype=np.float32),
            "W_qkv": np.ascontiguousarray(W_qkv, dtype=np.float32),
            "b_qkv": np.ascontiguousarray(b_qkv, dtype=np.float32),
        }
        for c in range(NCORE)
    ]
    res = run_bass_kernel_spmd(nc, in_maps, core_ids=list(range(NCORE)))
    return np.stack([res.results[c]["out"] for c in range(NCORE)], axis=0)


# revision 19
# speedup vs baseline: 1.0229x; 1.0229x over previous
"""Multi-head self-attention Trainium2 kernel (8 NeuronCores, batch-parallel).

Reference: qkv = x @ W_qkv + b; 12-head scaled-dot-product attention; concat.
Shapes: x[8,1024,768], W_qkv[768,2304], b_qkv[2304] -> out[8,1024,768].
Sharding: one batch element per core; W/b replicated to all cores.

Per-core dataflow:
  x --PE transpose--> xT[768,1024] (f32r), copies batched 4 chunks at a time
  qk tiles (bf16): per (f-block, token-half) [128,512] = W-block(lhsT) @ xT
    produced in N>=256 slices; Q/K biases added on the PSUM->SBUF copy
  V[128,12,66] bf16 per token chunk (strips of 4 heads; col 64 = ones)
  per (pair p, q-half qh), per key-chunk kc:
    scT[128,2,512] = K-slice(lhsT) @ Q-half  (2 row-tiled MMs, one per head)
    ex[128,2,512] bf16 = ACT Exp(0.125 * scT)   (scale folded into ACT)
    av[q=128,65] += ex-chunk(lhsT) @ [V_h|1]  bf16 N=65 MMs, accumulated
      over kc; av already in [q, feature] orientation, col 64 = denominator
  normalize: rc = 1/av[:,:,64] (DVE), onat[:, c, h*64:...] = av * rc
  out DMA per chunk once the last pair finishes its q-half.

Scheduling: W is DMA'd in priority order (pair-0 Q/K columns, V strip 0,
then later pairs); QK-tile and V-strip production is spread across the
pair loop just-in-time so PE work per key-chunk stays balanced against
the ACT exp stream (ACT is the co-bottleneck at ~1038ns per key-chunk).
"""

import contextlib
import json as _json

import numpy as np

import concourse.bass as bass
import concourse.mybir as mybir
import concourse.tile as tile
from concourse.bass_utils import run_bass_kernel_spmd
from concourse.masks import make_identity

# --- BIR sync-wait legalization ------------------------------------------
# walrus's codegen in this toolchain accepts only one sync-wait command per
# instruction. Split every multi-wait instruction into N-1 preceding
# single-wait EventSemaphore instructions on the same engine.


def _legalize_sync_waits(bir_json: bytes) -> bytes:
    m = _json.loads(bir_json)
    ctr = 0
    for fn in m["functions"]:
        for bb in fn["blocks"]:
            out = []
            for ins in bb["instructions"]:
                si = ins.get("sync_info")
                waits = si.get("on_wait", []) if si else []
                if len(waits) > 1:
                    for w in waits[:-1]:
                        ctr += 1
                        out.append(
                            {
                                "debug": ins.get("debug", 0),
                                "engine": ins["engine"],
                                "ins": [],
                                "outs": [],
                                "name": f"evw-split-{ctr}",
                                "opcode": "EventSemaphore",
                                "sync_info": {"on_update": [], "on_wait": [w]},
                            }
                        )
                    si["on_wait"] = [waits[-1]]
                out.append(ins)
            bb["instructions"] = out
    return _json.dumps(m).encode()


_fixup_installed = False


def _install_bir_fixup():
    global _fixup_installed
    if _fixup_installed:
        return
    _fixup_installed = True
    import concourse.bass_utils as _bu

    _orig = _bu.compile_bir_kernel

    def _patched(bir_json, tmpdir, neff_name="file.neff"):
        if isinstance(bir_json, str):
            bir_json = bir_json.encode()
        return _orig(_legalize_sync_waits(bir_json), tmpdir, neff_name)

    _bu.compile_bir_kernel = _patched
    try:
        import concourse.bass2jax as _b2j

        _b2j.compile_bir_kernel = _patched
    except ImportError:
        pass


_install_bir_fixup()

B, N, D, H = 8, 1024, 768, 12
HD = D // H            # 64
F3 = 3 * D             # 2304
NCORE = 8
P = 128
NCHUNK = N // P        # 8 token chunks
KD = D // P            # 6 d_in chunks
QH = 512               # q-half size
NPAIR = H // 2         # 6
VW = HD + 1            # 65 (V cols + denominator ones col)
VPAD = 66              # padded per-head V width (4-byte aligned bf16)

f32 = mybir.dt.float32
f32r = mybir.dt.float32r
bf16 = mybir.dt.bfloat16
fp8 = mybir.dt.float8e4
FT = mybir.ActivationFunctionType
ALU = mybir.AluOpType
DR = mybir.MatmulPerfMode.DoubleRow

# fp8 scores were tried and rejected: DoubleRow's AP layout crashed the
# device for Ki=32, and fp8e4m3's ~3.6%/operand quantization error puts the
# output at ~1-2e-2 — too close to the 2e-2 gate. Keep bf16.
SCORES_FP8 = False


def build_attention_nc():
    nc = bass.Bass()
    x_d = nc.declare_dram_parameter("x", [N, D], f32, isOutput=False)
    w_d = nc.declare_dram_parameter("W_qkv", [D, F3], f32, isOutput=False)
    b_d = nc.declare_dram_parameter("b_qkv", [F3], f32, isOutput=False)
    o_d = nc.declare_dram_parameter("out", [N, D], f32, isOutput=True)

    with tile.TileContext(nc) as tc, contextlib.ExitStack() as ctx:
        singles = ctx.enter_context(tc.tile_pool(name="singles", bufs=1))
        qkpool = ctx.enter_context(tc.tile_pool(name="qkpool", bufs=10))
        vpool = ctx.enter_context(tc.tile_pool(name="vpool", bufs=NCHUNK))
        exppool = ctx.enter_context(tc.tile_pool(name="exppool", bufs=5))
        recpool = ctx.enter_context(tc.tile_pool(name="recpool", bufs=4))

        # PSUM budget (8 banks): wk [128,512] x2 = 2; sc [128,2,512] x2 = 4;
        # av [128,4,66] x2 = 2.
        wkps = ctx.enter_context(tc.tile_pool(name="wkps", bufs=2, space="PSUM"))
        scps = ctx.enter_context(tc.tile_pool(name="scps", bufs=2, space="PSUM"))
        avps = ctx.enter_context(tc.tile_pool(name="avps", bufs=2, space="PSUM"))

        def wk_psum():
            return wkps.tile([P, QH], f32, tag="wk", name="wktile")

        # ------------- constants -------------------------------------------
        ident = singles.tile([P, P], f32)
        make_identity(nc, ident)  # gpsimd

        ident_r = singles.tile([P, P], f32r)
        nc.vector.tensor_copy(out=ident_r, in_=ident)

        ones_f32 = singles.tile([P, 1], f32)
        nc.vector.memset(ones_f32, 1.0)
        ones_row_st = singles.tile([1, P], f32)
        nc.vector.memset(ones_row_st, 1.0)
        ones_row = singles.tile([1, P], f32r)
        nc.vector.tensor_copy(out=ones_row, in_=ones_row_st)

        # dummy exp to trigger the ACT table load early
        actwarm = singles.tile([1, 2], f32)
        nc.vector.memset(actwarm, 0.0)
        nc.scalar.activation(actwarm, actwarm, FT.Exp)

        bv_st = singles.tile([1, D], f32)
        nc.sync.dma_start(out=bv_st, in_=b_d[2 * D : 3 * D][None, :])
        bv_sb = singles.tile([1, D], f32r)
        nc.vector.tensor_copy(out=bv_sb, in_=bv_st)

        # ------------- input DMAs (batched, priority order) ----------------
        # HWDGE charges a flat ~625ns per DMA instruction, serialized — so
        # batch: one DMA per W column block covering all 6 k-chunks, and
        # 2-chunk x DMAs.
        x_big = singles.tile([P, NCHUNK, D], f32r)
        x_sb = [x_big[:, c, :] for c in range(NCHUNK)]

        def dma_x(c0, nc_=2):
            nc.sync.dma_start(
                out=x_big[:, c0 : c0 + nc_, :],
                in_=x_d[c0 * P : (c0 + nc_) * P, :]
                .bitcast(f32r)
                .rearrange("(c p) d -> p c d", p=P),
            )

        w_big = singles.tile([P, KD, F3], f32r)
        w_sb = [w_big[:, k, :] for k in range(KD)]

        def dma_w_cols(f0, fw):
            nc.sync.dma_start(
                out=w_big[:, :, f0 : f0 + fw],
                in_=w_d[:, f0 : f0 + fw]
                .bitcast(f32r)
                .rearrange("(k p) f -> p k f", p=P),
            )

        # single-chunk x DMAs at the start so transposes begin ASAP and the
        # PE stays continuously busy through its ramp-up; K cols + bias
        # before Q cols so kt production (needing only x0,x1) starts first
        for c in range(4):
            dma_x(c, 1)
        dma_w_cols(0 * P, P)          # pair-0 Q cols
        dma_w_cols(6 * P, P)          # pair-0 K cols
        b_sb = singles.tile([P, 2 * KD], f32)  # Q/K biases only; V uses bv
        nc.sync.dma_start(
            out=b_sb, in_=b_d[0 : 2 * D].rearrange("(t p) -> p t", p=P)
        )
        dma_w_cols(2 * D, 2 * P)      # V strip 0 (heads 0-1, 2-3)
        dma_x(4)
        dma_x(6)
        dma_w_cols(1 * P, P)          # pair-1 Q
        dma_w_cols(7 * P, P)          # pair-1 K
        dma_w_cols(2 * D + 2 * P, 2 * P)   # V strip 1 (heads 4-7)
        dma_w_cols(2 * P, P)
        dma_w_cols(8 * P, P)
        dma_w_cols(2 * D + 4 * P, 2 * P)   # V strip 2 (heads 8-11)
        for p in range(3, NPAIR):
            dma_w_cols(p * P, P)
            dma_w_cols((6 + p) * P, P)

        # ------------- x^T (PE transposes, batched copies) ------------------
        # xt is one [P, KD, N] tile so a chunk's transposes for several
        # k-slices drain through a single strided DVE copy
        xt_big = singles.tile([P, KD, N], f32r)
        xt = [xt_big[:, k, :] for k in range(KD)]
        xt8_big = singles.tile([P, KD, N], bf16)
        w8v = singles.tile([P, KD, D], bf16)

        def transpose_chunk(c):
            # transpose x chunk c into xt[k][:, c*P:(c+1)*P] for all k
            for k0, kn in ((0, 4), (4, 2)):
                ps = wk_psum()[:, 0 : kn * P]
                for j in range(kn):
                    nc.tensor.transpose(
                        ps[:, j * P : (j + 1) * P].bitcast(f32r),
                        x_sb[c][:, (k0 + j) * P : (k0 + j + 1) * P],
                        ident_r,
                    )
                nc.vector.tensor_copy(
                    out=xt_big[:, k0 : k0 + kn, c * P : (c + 1) * P],
                    in_=ps.rearrange("p (k q) -> p k q", q=P).bitcast(f32r),
                )
            # bf16 shadow of xt for V production, on the idle Pool engine
            nc.gpsimd.tensor_copy(
                out=xt8_big[:, :, c * P : (c + 1) * P],
                in_=xt_big[:, :, c * P : (c + 1) * P],
            )

        # broadcast b_v across partitions once: bvb[p, f] = b_v[f]
        bvb = singles.tile([P, D], f32)
        for f0 in range(0, D, 256):
            ps = wk_psum()[:, 0:256]
            nc.tensor.matmul(
                ps, ones_row, bv_sb[:, f0 : f0 + 256], start=True, stop=True
            )
            nc.vector.tensor_copy(out=bvb[:, f0 : f0 + 256], in_=ps)

        # ------------- qk tiles ---------------------------------------------
        # qk[(f, half)]: [128, 512] bf16; partitions = features f*128..+128,
        # cols = tokens half*512..+512. f 0..5 = Q blocks, 6..11 = K blocks.
        qk_tiles = {}
        qk8_tiles = {}
        qk_dt = fp8 if SCORES_FP8 else bf16

        def get_qk(f, half):
            key = (f, half)
            if key not in qk_tiles:
                qk_tiles[key] = qkpool.tile(
                    [P, QH], qk_dt, tag="qk", name=f"qk{f}_{half}"
                )
            return qk_tiles[key]

        def get_qk8(f, half):
            # DoubleRow layout: partition 32*hi+p', free (g, tok) holds
            # feature 64*hi + 32*g + p' of block f
            key = (f, half)
            if key not in qk8_tiles:
                qk8_tiles[key] = qkpool.tile(
                    [64, 2, QH], fp8, tag="qk8", name=f"qk8_{f}_{half}"
                )
            return qk8_tiles[key]

        def make_qk(f, half, n0=0, nw=QH, ks=0, ke=KD, _ps=[None]):
            # produce token-cols [n0, n0+nw) of tile (f, half); nw >= 256.
            # ks/ke allow k-chunk-split emission (jit pacing); the PSUM tile
            # is carried across the split via _ps.
            t = get_qk(f, half)
            if ks == 0:
                _ps[0] = wk_psum()[:, 0:nw]
            ps = _ps[0]
            for k in range(ks, ke):
                nc.tensor.matmul(
                    ps,
                    w_sb[k][:, f * P : (f + 1) * P],
                    xt[k][:, half * QH + n0 : half * QH + n0 + nw],
                    start=(k == 0),
                    stop=(k == KD - 1),
                )
            if ke == KD:
                nc.vector.tensor_scalar_add(
                    t[:, n0 : n0 + nw], ps, b_sb[:, f : f + 1]
                )
                if SCORES_FP8:
                    # cross-partition remap into the DoubleRow layout:
                    # out(32*hi+p', g, n) <- t(64*hi + 32*g + p', n)
                    t8 = get_qk8(f, half)
                    for hi in range(2):
                        nc.sync.dma_start(
                            out=t8[32 * hi : 32 * hi + 32, :, n0 : n0 + nw],
                            in_=t[64 * hi : 64 * hi + 64, n0 : n0 + nw]
                            .rearrange("(g q) n -> q g n", g=2),
                        )
            return t

        def qk_halves(f, half):
            # two pacing thunks producing tile (f, half) split by k-chunks
            return [
                lambda: make_qk(f, half, ks=0, ke=3),
                lambda: make_qk(f, half, ks=3, ke=KD),
            ]

        # ------------- V tiles ----------------------------------------------
        # v[c]: [128, 12, 66] bf16; [:, h, 0:64] = V for head h, [:, h, 64] = 1
        v_sb = []
        for c in range(NCHUNK):
            t = vpool.tile([P, H, VPAD], bf16, tag="v", name=f"v{c}")
            v_sb.append(t)

        def conv_w8v(s):
            # bf16 shadow of the V-weight strip pair 2s/2s+1, on Pool
            nc.gpsimd.tensor_copy(
                out=w8v[:, :, s * 256 : (s + 1) * 256],
                in_=w_big[:, :, 2 * D + s * 256 : 2 * D + (s + 1) * 256],
            )

        def make_v0(c):
            # heads 0-3 (pairs 0-1) via the f32r N=256 path: available as
            # soon as the Vs0 weights land, no bf16 conversion in the way
            nc.vector.tensor_copy(
                out=v_sb[c][:, :, HD : HD + 1],
                in_=ones_f32[:, 0:1, None].to_broadcast([P, H, 1]),
            )
            ps = wk_psum()[:, 0:256]
            for k in range(KD):
                nc.tensor.matmul(
                    ps,
                    xt[k][:, c * P : (c + 1) * P],
                    w_sb[k][:, 2 * D : 2 * D + 256],
                    start=(k == 0),
                    stop=(k == KD - 1),
                )
            nc.vector.tensor_tensor(
                v_sb[c][:, 0:4, 0:HD],
                ps.rearrange("p (h d) -> p h d", d=HD),
                bvb[:, 0:256].rearrange("p (h d) -> p h d", d=HD),
                ALU.add,
            )

        def make_v(c, p):
            # V strip for pair p (heads 2p, 2p+1), chunk c; bf16 inputs keep
            # the N=128 matmuls at 1 cycle/row so strips can be produced
            # just-in-time per pair (filling the tail pairs' PE idle)
            f0 = p * P
            ps = wk_psum()[:, 0:P]
            for k in range(KD):
                nc.tensor.matmul(
                    ps,
                    xt8_big[:, k, c * P : (c + 1) * P],
                    w8v[:, k, f0 : f0 + P],
                    start=(k == 0),
                    stop=(k == KD - 1),
                )
            nc.vector.tensor_tensor(
                v_sb[c][:, 2 * p : 2 * p + 2, 0:HD],
                ps.rearrange("p (h d) -> p h d", d=HD),
                bvb[:, f0 : f0 + P].rearrange("p (h d) -> p h d", d=HD),
                ALU.add,
            )

        # ------------- bootstrap: transposes + first tiles ------------------
        for c in range(4):
            transpose_chunk(c)

        make_qk(0, 0)                   # qt(pair0, qh0)
        make_qk(6, 0, 0, 256)           # kt(pair0) tokens 0:256

        onat = singles.tile([P, NCHUNK, D], f32)

        # pair-0-qh0 slots are handcrafted against DMA arrival order (jit
        # runs before each iteration's sc MMs, so an item may produce the
        # tile that same iteration reads).
        slot_items = {
            0: [lambda: make_qk(6, 0, 256, 256)],
            1: [lambda: transpose_chunk(4)],
            2: [lambda: transpose_chunk(5)],
            3: [lambda: make_qk(6, 1, 0, 256)],
            4: [lambda: transpose_chunk(6)],
            5: [lambda: transpose_chunk(7)],
            6: [lambda: make_qk(6, 1, 256, 256)],
            7: [qk_halves(0, 1)[0]],
            8: [qk_halves(0, 1)[1]],
        }

        # Remaining production work is placed in the LATEST legal slot
        # (backward greedy by deadline) so the per-iteration PE load is
        # spread toward uniform: the tail pairs have no natural projection
        # work and would otherwise leave the PE idle while ACT catches up.
        items = []  # (deadline_slot, thunk)
        for p in range(1, NPAIR):
            for half in range(2):
                for th in qk_halves(6 + p, half):
                    items.append((16 * p + 4 * half, th))
            for qh in range(2):
                for th in qk_halves(p, qh):
                    items.append((16 * p + 8 * qh, th))
        for vp in range(2, NPAIR):
            for c in range(NCHUNK):
                items.append((16 * vp + c, lambda c=c, vp=vp: make_v(c, vp)))

        free_slots = [s for s in range(9, 16 * NPAIR) if s not in slot_items]
        for s in reversed(free_slots):
            best = None
            for i, (dl, th) in enumerate(items):
                # ties pick the later list index so split-production halves
                # keep their emission order (h1 before h2)
                if dl >= s and (best is None or dl >= items[best][0]):
                    best = i
            if best is not None:
                slot_items[s] = [items.pop(best)[1]]
        for s in reversed(free_slots):  # anything left: earliest slots
            if s not in slot_items and items:
                slot_items[s] = [items.pop()[1]]
        assert not items, f"unscheduled jit items: {len(items)}"
        # Pool-engine bf16 weight conversions, ordered by DMA arrival
        slot_items.setdefault(8, []).insert(0, lambda: conv_w8v(1))
        slot_items.setdefault(14, []).insert(0, lambda: conv_w8v(2))

        # ------------- attention pair loop ----------------------------------
        def normalize(p, qh, av):
            # rc = 1/denominator, onat = av * rc
            for hi in range(2):
                h = 2 * p + hi
                rc = recpool.tile([P, 4], f32, tag="rec", name="rc")
                nc.vector.reciprocal(out=rc, in_=av[hi][:, :, HD])
                nc.vector.tensor_tensor(
                    onat[:, qh * 4 : (qh + 1) * 4, h * HD : (h + 1) * HD],
                    av[hi][:, :, 0:HD],
                    rc[:, :, None].to_broadcast([P, 4, HD]),
                    ALU.mult,
                )
                if p == NPAIR - 1 and qh == 1:
                    # final-tail head: DMA its 64 columns immediately
                    nc.sync.dma_start(
                        out=o_d[4 * P : NCHUNK * P, h * HD : (h + 1) * HD]
                        .rearrange("(c p) f -> p c f", p=P),
                        in_=onat[:, 4:NCHUNK, h * HD : (h + 1) * HD],
                    )
            if p == NPAIR - 2 and qh == 1:
                # heads 0-9 of chunks 4-7 are final: DMA them now so only
                # the last pair's 128 columns remain for the tail
                nc.sync.dma_start(
                    out=o_d[4 * P : NCHUNK * P, 0 : 5 * P].rearrange(
                        "(c p) f -> p c f", p=P
                    ),
                    in_=onat[:, 4:NCHUNK, 0 : 5 * P],
                )
            if p == NPAIR - 1 and qh == 0:
                nc.sync.dma_start(
                    out=o_d[0 : 4 * P, :].rearrange("(c p) f -> p c f", p=P),
                    in_=onat[:, 0:4, :],
                )

        # software pipeline: AV for iteration i emitted during iteration i+1,
        # so the next sc MMs (and the exp they feed) aren't serialized behind
        # the AV tail at q-half boundaries.
        stream = [
            (p, qh, kc)
            for p in range(NPAIR)
            for qh in range(2)
            for kc in range(NCHUNK)
        ]
        av_cur = None
        deferred = None  # (p, qh, kc, av, ex) AV work from previous iteration

        def emit_av(p, qh, kc, av, ex):
            for hi in range(2):
                for qc in range(4):
                    # one bank-clearing start per av bank: later first-writes
                    # overwrite per-element (has_written cleared by the
                    # start), later kc's accumulate
                    nc.tensor.matmul(
                        av[hi][:, qc, 0:VW],
                        ex[:, hi, qc * P : (qc + 1) * P],
                        v_sb[kc][:, 2 * p + hi, 0:VW],
                        start=(kc == 0 and qc == 0),
                        stop=(kc == NCHUNK - 1 and qc == 3),
                        skip_group_check=True,
                    )

        for it, (p, qh, kc) in enumerate(stream):
            for th in slot_items.get(it, ()):
                th()
            kcol = (kc % 4) * P
            sc = scps.tile([P, 2, QH], f32, tag="sc", name="sc")
            if SCORES_FP8:
                qt8 = get_qk8(p, qh)
                kt8 = get_qk8(6 + p, kc // 4)
                for hi in range(2):
                    nc.tensor.matmul(
                        sc[:, hi, :],
                        kt8[32 * hi : 32 * hi + 32, :, kcol : kcol + P],
                        qt8[32 * hi : 32 * hi + 32, :, :],
                        start=True,
                        stop=True,
                        perf_mode=DR,
                    )
            else:
                qt = get_qk(p, qh)
                kth = get_qk(6 + p, kc // 4)
                for hi in range(2):
                    nc.tensor.matmul(
                        sc[:, hi, :],
                        kth[64 * hi : 64 * hi + 64, kcol : kcol + P],
                        qt[64 * hi : 64 * hi + 64, :],
                        start=True,
                        stop=True,
                        tile_position=(64 * hi, 0),
                    )
            if p == 0 and qh == 0:
                make_v0(kc)  # heads 0-3 JIT (AV needs it next iteration)
            ex = exppool.tile([P, 2, QH], bf16, tag="exp", name="ex")
            nc.scalar.activation(ex, sc, FT.Exp, scale=0.125)
            if deferred is not None:
                emit_av(*deferred)
                dp, dqh, dkc, dav, _ = deferred
                if dkc == NCHUNK - 1:
                    normalize(dp, dqh, dav)
            if kc == 0:
                av_cur = [
                    avps.tile([P, 4, VPAD], f32, tag="av", name=f"av{hi}")
                    for hi in range(2)
                ]
            deferred = (p, qh, kc, av_cur, ex)
        emit_av(*deferred)
        normalize(NPAIR - 1, 1, av_cur)

    return nc


def kernel(x: np.ndarray, W_qkv: np.ndarray, b_qkv: np.ndarray) -> np.ndarray:
    nc = build_attention_nc()
    in_maps = [
        {
            "x": np.ascontiguousarray(x[c], dtype=np.float32),
            "W_qkv": np.ascontiguousarray(W_qkv, dtype=np.float32),
            "b_qkv": np.ascontiguousarray(b_qkv, dtype=np.float32),
        }
        for c in range(NCORE)
    ]
    res = run_bass_kernel_spmd(nc, in_maps, core_ids=list(range(NCORE)))
    return np.stack([res.results[c]["out"] for c in range(NCORE)], axis=0)
